# revision 7
# baseline (speedup 1.0000x reference)
"""Trainium2 Bass kernel for nn_Decoder_34351148434122 (decoder layer).

Sharding: 8 cores = 2 batches x 4 token-blocks of 512 tokens. Each core
computes the full output rows for its token block; no cross-core collectives.

Per-core dataflow (transposed-canonical, fp32r matmuls; MLP in bf16):
  h1T [c,t] from host-pretransposed xT + on-device RMSNorm (ones-matmul for
  the cross-partition sum, PE broadcast of the scale). vT/kT/gateT via
  weight-stationary matmuls; RoPE on v in natural layout; scores [s,t] via PE
  (kT stationary, vT moving); softmax in-place (max-subtract, fused
  Exp+row-sum); pT via PE transpose; attnT = v @ pT; aT = attnT_rep *
  sigmoid(gateT); x2 = x + aT.T @ Wo; MLP: h3T (bf16) from x2T, gT/uT
  weight-stationary bf16, m = silu(g)*u, out = x2 + m.T @ Wd.

The core's own token block is rotated to t-block 0 by the host so one SPMD
program works for all cores (k/gate projections always read block 0's h1T);
mask columns and rope tables follow the same t permutation.
"""
import os
import numpy as np
from contextlib import ExitStack

import ml_dtypes
import concourse.bass as bass
import concourse.mybir as mybir
import concourse.tile as tile
from concourse import bacc
from concourse.bass_utils import run_bass_kernel_spmd

F32 = mybir.dt.float32
F32R = mybir.dt.float32r
BF16 = mybir.dt.bfloat16
MUL = mybir.AluOpType.mult
ADD = mybir.AluOpType.add
SUB = mybir.AluOpType.subtract
AF = mybir.ActivationFunctionType

B, S, HID = 2, 2048, 2048
NH, NKV, HD = 16, 4, 128
FF = 8192
EPS = 1e-6
SB = 512           # tokens per core
NT = HID // 128    # 16 c-tiles
NFT = FF // 128    # 64 ff-tiles
CSC = float(HD) ** 0.5

_cache = {}


def _build():
    nc = bacc.Bacc("TRN2", target_bir_lowering=False, debug=False, num_devices=8)

    d_xT = nc.dram_tensor("xT", [4, NT, 128, SB], F32, kind="ExternalInput")
    d_xq = nc.dram_tensor("xq", [4, 128, HID], F32, kind="ExternalInput")
    d_mask = nc.dram_tensor("maskq", [4, 128, S], F32, kind="ExternalInput")
    d_cosA = nc.dram_tensor("cosA", [16, 128, 128], F32, kind="ExternalInput")
    d_cosB = nc.dram_tensor("cosB", [16, 128, 128], F32, kind="ExternalInput")
    d_sinA = nc.dram_tensor("sinA", [16, 128, 128], F32, kind="ExternalInput")
    d_sinB = nc.dram_tensor("sinB", [16, 128, 128], F32, kind="ExternalInput")
    d_ln1 = nc.dram_tensor("ln1t", [128, NT], F32, kind="ExternalInput")
    d_ln2 = nc.dram_tensor("ln2t", [128, NT], F32, kind="ExternalInput")
    d_onec = nc.dram_tensor("onec", [128, 1], F32, kind="ExternalInput")
    d_oner = nc.dram_tensor("oner", [1, 128], F32, kind="ExternalInput")
    d_ident = nc.dram_tensor("ident", [128, 128], F32, kind="ExternalInput")
    d_wv = nc.dram_tensor("wvp", [NT, 128, 512], F32, kind="ExternalInput")
    d_wk = nc.dram_tensor("wkp", [NT, 128, 512], F32, kind="ExternalInput")
    d_wqg = nc.dram_tensor("wqgp", [NT, 128, NT, 128], F32, kind="ExternalInput")
    d_wo = nc.dram_tensor("wop", [NT, 128, HID], F32, kind="ExternalInput")
    d_wg = nc.dram_tensor("wgp", [NFT, 128, NT, 128], BF16, kind="ExternalInput")
    d_wu = nc.dram_tensor("wup", [NFT, 128, NT, 128], BF16, kind="ExternalInput")
    d_wd = nc.dram_tensor("wdp", [NFT, 128, HID], BF16, kind="ExternalInput")
    d_out = nc.dram_tensor("out", [4, 128, HID], F32, kind="ExternalOutput")

    with tile.TileContext(nc) as tc:
        es_const = ExitStack()
        cpool = es_const.enter_context(tc.tile_pool(name="const", bufs=1))

        onec = cpool.tile([128, 1], F32R, tag="onec")
        oner = cpool.tile([1, 128], F32R, tag="oner")
        identf = cpool.tile([128, 128], F32, tag="identf")
        epsT = cpool.tile([1, 1], F32, tag="epsT")
        nc.vector.memset(epsT[:], EPS)
        ln1 = cpool.tile([128, NT], F32, tag="ln1")
        ln2 = cpool.tile([128, NT], F32, tag="ln2")
        nc.sync.dma_start(onec[:], d_onec[:].bitcast(F32R))
        nc.sync.dma_start(oner[:], d_oner[:].bitcast(F32R))
        nc.sync.dma_start(identf[:], d_ident[:])
        nc.sync.dma_start(ln1[:], d_ln1[:])
        nc.sync.dma_start(ln2[:], d_ln2[:])

        # persistent A->B
        es_ab = ExitStack()
        pab = es_ab.enter_context(tc.tile_pool(name="pab", bufs=1))
        v_sb = pab.tile([128, NT, 512], F32R, tag="v")    # [t%128, g, 4h*128d]
        kT = pab.tile([128, NKV, SB], F32R, tag="kT")     # [d, h, s]
        sg = pab.tile([128, NT, SB], F32R, tag="sg")      # [f%128, ft, s]
        # shared-slot big tiles (disjoint lifetimes, LIFO-safe):
        #   tag h1aT: h1T (A, per t-block) -> aT (B->C) -> x2T (D)
        #   tag x2pT: pT (B, per head)     -> x2 (C->D)

        # ============ Phase A: norm1 + projections ============
        es_a = ExitStack()
        pa2 = es_a.enter_context(tc.tile_pool(name="pa2", bufs=2))
        pa3 = es_a.enter_context(tc.tile_pool(name="pa3", bufs=3))
        psA = es_a.enter_context(tc.tile_pool(name="psA", bufs=1, space="PSUM"))

        for tb in range(4):
            # pass 1: sum of squares (ones-matmul over c-tiles)
            pssq = psA.tile([1, SB], F32, tag="s1")
            for ct in range(NT):
                xct = pa3.tile([128, SB], F32, tag="xct")
                nc.sync.dma_start(xct[:], d_xT[tb, ct])
                sq = pa3.tile([128, SB], F32R, tag="sq")
                nc.vector.tensor_tensor(out=sq[:], in0=xct[:], in1=xct[:], op=MUL)
                nc.tensor.matmul(pssq[:], onec[:], sq[:],
                                 start=(ct == 0), stop=(ct == NT - 1))
            sqr = cpool.tile([1, SB], F32, tag="sqr")
            nc.scalar.activation(sqr[:], pssq[:], AF.Sqrt,
                                 bias=epsT[:], scale=1.0 / HID)
            rsc = cpool.tile([1, SB], F32, tag="rsc")
            nc.vector.reciprocal(rsc[:], sqr[:])
            rscr = cpool.tile([1, SB], F32R, tag="rscr")
            nc.vector.tensor_copy(rscr[:], rsc[:])
            pbcast = psA.tile([128, SB], F32, tag="b1")
            nc.tensor.matmul(pbcast[:], oner[:], rscr[:], start=True, stop=True)

            # pass 2: h1T[:, ct, :] = xT*scale*ln1 (re-DMA xT per c-tile)
            h1T = cpool.tile([128, NT, SB], F32R, tag="h1aT")
            for ct in range(NT):
                xct = pa3.tile([128, SB], F32, tag="xct")
                nc.sync.dma_start(xct[:], d_xT[tb, ct])
                htmp = pa3.tile([128, SB], F32, tag="htmp")
                nc.vector.tensor_tensor(out=htmp[:], in0=xct[:], in1=pbcast[:],
                                        op=MUL)
                nc.vector.tensor_scalar_mul(h1T[:, ct, :], htmp[:],
                                            ln1[:, ct:ct + 1])

            # v projection for this t-block (acts-stationary, 4 psum banks)
            pv = psA.tile([128, 4, 512], F32, tag="acc4")
            for ct in range(NT):
                wvt = pa3.tile([128, 512], F32R, tag="wvt")
                nc.sync.dma_start(wvt[:], d_wv[ct].bitcast(F32R))
                for tt in range(4):
                    nc.tensor.matmul(
                        pv[:, tt, :],
                        h1T[:, ct, tt * 128:(tt + 1) * 128], wvt[:],
                        start=(ct == 0), stop=(ct == NT - 1),
                        skip_group_check=True)
            # RoPE epilogue per t-tile -> v_sb[:, tb*4+tt, :]
            for tt in range(4):
                g = tb * 4 + tt
                ca = pa2.tile([128, 128], F32, tag="ca")
                cb = pa2.tile([128, 128], F32, tag="cb")
                sa = pa2.tile([128, 128], F32, tag="sa")
                sb_ = pa2.tile([128, 128], F32, tag="sb")
                nc.sync.dma_start(ca[:], d_cosA[g])
                nc.sync.dma_start(cb[:], d_cosB[g])
                nc.sync.dma_start(sa[:], d_sinA[g])
                nc.sync.dma_start(sb_[:], d_sinB[g])
                pv4 = pv[:, tt, :].rearrange("p (h d) -> p h d", h=NKV)
                v4 = v_sb[:, g, :].rearrange("p (h d) -> p h d", h=NKV)
                ca4 = ca[:].rearrange("p (h d) -> p h d", h=NKV)
                cb4 = cb[:].rearrange("p (h d) -> p h d", h=NKV)
                sa4 = sa[:].rearrange("p (h d) -> p h d", h=NKV)
                sb4 = sb_[:].rearrange("p (h d) -> p h d", h=NKV)
                t1 = pa2.tile([128, NKV, 32], F32, tag="rt1")
                t2 = pa2.tile([128, NKV, 32], F32, tag="rt2")
                nc.vector.tensor_tensor(out=t1[:], in0=pv4[:, :, 0:32],
                                        in1=ca4[:, :, 0:32], op=MUL)
                nc.vector.tensor_tensor(out=t2[:], in0=pv4[:, :, 32:64],
                                        in1=sa4[:, :, 0:32], op=MUL)
                nc.vector.tensor_tensor(out=v4[:, :, 0:32], in0=t1[:], in1=t2[:],
                                        op=SUB)
                nc.vector.tensor_tensor(out=t1[:], in0=pv4[:, :, 32:64],
                                        in1=cb4[:, :, 0:32], op=MUL)
                nc.vector.tensor_tensor(out=t2[:], in0=pv4[:, :, 0:32],
                                        in1=sb4[:, :, 0:32], op=MUL)
                nc.vector.tensor_tensor(out=v4[:, :, 32:64], in0=t1[:], in1=t2[:],
                                        op=ADD)
                nc.vector.tensor_copy(v4[:, :, 64:128], pv4[:, :, 64:128])

            if tb == 0:
                # own-block projections (host rotates blocks so tb0 == own)
                pk = psA.tile([128, 4, 512], F32, tag="acc4")
                for ct in range(NT):
                    wkt = pa3.tile([128, 512], F32R, tag="wkt")
                    nc.sync.dma_start(wkt[:], d_wk[ct].bitcast(F32R))
                    for h in range(NKV):
                        nc.tensor.matmul(
                            pk[:, h, :], wkt[:, h * 128:(h + 1) * 128],
                            h1T[:, ct, :],
                            start=(ct == 0), stop=(ct == NT - 1),
                            skip_group_check=True)
                nc.vector.tensor_copy(kT[:], pk[:])
                for ft in range(NT):
                    wqt = pa2.tile([128, NT, 128], F32R, tag="wqt")
                    nc.sync.dma_start(wqt[:], d_wqg[ft].bitcast(F32R))
                    pg = psA.tile([128, 512], F32, tag="pg1")
                    for ct in range(NT):
                        nc.tensor.matmul(pg[:], wqt[:, ct, :], h1T[:, ct, :],
                                         start=(ct == 0), stop=(ct == NT - 1))
                    nc.scalar.activation(sg[:, ft, :], pg[:], AF.Sigmoid)
        es_a.close()

        # ============ Phase B: attention ============
        aT = cpool.tile([128, NT, SB], F32R, tag="h1aT")

        es_b = ExitStack()
        pb = es_b.enter_context(tc.tile_pool(name="pb", bufs=2))
        pb1 = es_b.enter_context(tc.tile_pool(name="pb1", bufs=1))
        psB = es_b.enter_context(tc.tile_pool(name="psB", bufs=1, space="PSUM"))
        psBt = es_b.enter_context(
            tc.tile_pool(name="psBt", bufs=2, space="PSUM"))

        for h in range(NKV):
            vT = pb1.tile([128, NT, 128], F32R, tag="vT")
            for g in range(NT):
                ptr = psBt.tile([128, 128], F32, tag="tr")
                nc.tensor.transpose(
                    ptr[:], v_sb[:, g, h * 128:(h + 1) * 128].bitcast(F32),
                    identf[:])
                nc.vector.tensor_copy(vT[:, g, :], ptr[:])
            pT = cpool.tile([128, NT, SB], F32R, tag="x2pT")
            for st in range(4):
                ps = psB.tile([128, 4, 512], F32, tag="acc4")
                for tb in range(4):
                    nc.tensor.matmul(
                        ps[:, tb, :],
                        kT[:, h, st * 128:(st + 1) * 128],
                        vT[:].rearrange("p g d -> p (g d)")[:, tb * 512:(tb + 1) * 512],
                        start=True, stop=True, skip_group_check=True)
                srow = pb.tile([128, S], F32, tag="srow")
                mrow = pb.tile([128, S], F32, tag="mrow")
                nc.sync.dma_start(mrow[:], d_mask[st])
                nc.vector.tensor_tensor(
                    out=srow[:], in0=ps[:].rearrange("p a b -> p (a b)"),
                    in1=mrow[:], op=ADD)
                nm = pb.tile([128, 1], F32, tag="nm")
                nc.vector.reduce_max(nm[:], srow[:], axis=mybir.AxisListType.X,
                                     negate=True)
                bias = pb.tile([128, 1], F32, tag="bias")
                nc.vector.tensor_scalar_mul(bias[:], nm[:], CSC)
                rsum = pb.tile([128, 1], F32, tag="rsum")
                nc.scalar.activation(srow[:], srow[:], AF.Exp,
                                     bias=bias[:], scale=CSC, accum_out=rsum[:])
                rr = pb.tile([128, 1], F32, tag="rr")
                nc.vector.reciprocal(rr[:], rsum[:])
                nc.vector.tensor_scalar_mul(srow[:], srow[:], rr[:])
                for g in range(NT):
                    ptr2 = psBt.tile([128, 128], F32, tag="tr")
                    nc.tensor.transpose(ptr2[:], srow[:, g * 128:(g + 1) * 128],
                                        identf[:])
                    nc.vector.tensor_copy(
                        pT[:, g, st * 128:(st + 1) * 128], ptr2[:])
            pat = psB.tile([128, SB], F32, tag="b1")
            for g in range(NT):
                nc.tensor.matmul(pat[:], v_sb[:, g, h * 128:(h + 1) * 128],
                                 pT[:, g, :],
                                 start=(g == 0), stop=(g == NT - 1))
            for r in range(4):
                ft = 4 * h + r
                nc.vector.tensor_tensor(out=aT[:, ft, :], in0=pat[:],
                                        in1=sg[:, ft, :], op=MUL)
        es_b.close()
        es_ab.close()

        # ============ Phase C: Wo + residual ============
        x2 = cpool.tile([128, 4, HID], F32, tag="x2pT")

        es_c = ExitStack()
        pc = es_c.enter_context(tc.tile_pool(name="pc", bufs=3))
        psC = es_c.enter_context(tc.tile_pool(name="psC", bufs=1, space="PSUM"))
        for st in range(4):
            px2 = psC.tile([128, 4, 512], F32, tag="acc4")
            for ft in range(NT):
                wot = pc.tile([128, HID], F32R, tag="wot")
                nc.sync.dma_start(wot[:], d_wo[ft].bitcast(F32R))
                for hb in range(4):
                    nc.tensor.matmul(
                        px2[:, hb, :],
                        aT[:, ft, st * 128:(st + 1) * 128],
                        wot[:, hb * 512:(hb + 1) * 512],
                        start=(ft == 0), stop=(ft == NT - 1),
                        skip_group_check=True)
            xqt = pc.tile([128, HID], F32, tag="xqt")
            nc.sync.dma_start(xqt[:], d_xq[st])
            nc.vector.tensor_tensor(out=x2[:, st, :],
                                    in0=px2[:].rearrange("p a b -> p (a b)"),
                                    in1=xqt[:], op=ADD)
        es_c.close()

        # ============ Phase D: MLP ============
        es_d = ExitStack()
        pdm = es_d.enter_context(tc.tile_pool(name="pdm", bufs=1))
        pd2 = es_d.enter_context(tc.tile_pool(name="pd2", bufs=2))
        pd3 = es_d.enter_context(tc.tile_pool(name="pd3", bufs=2))

        # x2T (bf16) via PE transpose + norm2 -> h3T (bf16)
        x2T = cpool.tile([128, NT, SB], BF16, tag="h1aT")
        h3T = pdm.tile([128, NT, SB], BF16, tag="h3T")
        m_sb = pdm.tile([128, NFT, SB], BF16, tag="m")

        es_d1 = ExitStack()
        psD1 = es_d1.enter_context(
            tc.tile_pool(name="psD1", bufs=2, space="PSUM"))
        psD1b = es_d1.enter_context(
            tc.tile_pool(name="psD1b", bufs=1, space="PSUM"))
        for st in range(4):
            for ct in range(NT):
                ptx = psD1.tile([128, 128], F32, tag="tr")
                nc.tensor.transpose(
                    ptx[:], x2[:, st, ct * 128:(ct + 1) * 128], identf[:])
                nc.vector.tensor_copy(x2T[:, ct, st * 128:(st + 1) * 128], ptx[:])
        pssq2 = psD1b.tile([1, SB], F32, tag="s1")
        for ct in range(NT):
            sq2 = pd3.tile([128, SB], F32R, tag="sq2")
            nc.vector.tensor_tensor(out=sq2[:], in0=x2T[:, ct, :],
                                    in1=x2T[:, ct, :], op=MUL)
            nc.tensor.matmul(pssq2[:], onec[:], sq2[:],
                             start=(ct == 0), stop=(ct == NT - 1))
        sqr2 = cpool.tile([1, SB], F32, tag="sqr")
        nc.scalar.activation(sqr2[:], pssq2[:], AF.Sqrt, bias=epsT[:],
                             scale=1.0 / HID)
        rsc2 = cpool.tile([1, SB], F32, tag="rsc")
        nc.vector.reciprocal(rsc2[:], sqr2[:])
        rscr2 = cpool.tile([1, SB], F32R, tag="rscr")
        nc.vector.tensor_copy(rscr2[:], rsc2[:])
        pbc2 = psD1b.tile([128, SB], F32, tag="b1")
        nc.tensor.matmul(pbc2[:], oner[:], rscr2[:], start=True, stop=True)
        for ct in range(NT):
            htmp2 = pd3.tile([128, SB], F32, tag="htmp2")
            nc.vector.tensor_tensor(out=htmp2[:], in0=x2T[:, ct, :],
                                    in1=pbc2[:], op=MUL)
            nc.vector.tensor_scalar_mul(h3T[:, ct, :], htmp2[:], ln2[:, ct:ct + 1])
        es_d1.close()

        # Wg/Wu -> m = silu(g)*u
        es_d2 = ExitStack()
        psD2 = es_d2.enter_context(
            tc.tile_pool(name="psD2", bufs=2, space="PSUM"))
        for pt in range(NFT):
            wgt = pd2.tile([128, NT, 128], BF16, tag="wgt")
            wut = pd2.tile([128, NT, 128], BF16, tag="wut")
            nc.sync.dma_start(wgt[:], d_wg[pt])
            nc.sync.dma_start(wut[:], d_wu[pt])
            pg2 = psD2.tile([128, SB], F32, tag="g1")
            pu2 = psD2.tile([128, SB], F32, tag="u1")
            for ct in range(NT):
                nc.tensor.matmul(pg2[:], wgt[:, ct, :], h3T[:, ct, :],
                                 start=(ct == 0), stop=(ct == NT - 1))
            for ct in range(NT):
                nc.tensor.matmul(pu2[:], wut[:, ct, :], h3T[:, ct, :],
                                 start=(ct == 0), stop=(ct == NT - 1))
            sil = pd3.tile([128, SB], F32, tag="sil")
            nc.scalar.activation(sil[:], pg2[:], AF.Silu)
            nc.vector.tensor_tensor(out=m_sb[:, pt, :], in0=sil[:], in1=pu2[:],
                                    op=MUL)
        es_d2.close()

        # Wd + residual; two st at a time (8 psum banks)
        es_d3 = ExitStack()
        psD3 = es_d3.enter_context(
            tc.tile_pool(name="psD3", bufs=1, space="PSUM"))
        for half in range(2):
            poA = psD3.tile([128, 4, 512], F32, tag="acc4a")
            poB = psD3.tile([128, 4, 512], F32, tag="acc4b")
            pos = (poA, poB)
            for pt in range(NFT):
                wdt = pd3.tile([128, HID], BF16, tag="wdt")
                nc.sync.dma_start(wdt[:], d_wd[pt])
                for si in range(2):
                    st = half * 2 + si
                    for hb in range(4):
                        nc.tensor.matmul(
                            pos[si][:, hb, :],
                            m_sb[:, pt, st * 128:(st + 1) * 128],
                            wdt[:, hb * 512:(hb + 1) * 512],
                            start=(pt == 0), stop=(pt == NFT - 1),
                            skip_group_check=True)
            for si in range(2):
                st = half * 2 + si
                ot = pd2.tile([128, HID], F32, tag="ot")
                nc.vector.tensor_tensor(
                    out=ot[:], in0=pos[si][:].rearrange("p a b -> p (a b)"),
                    in1=x2[:, st, :], op=ADD)
                nc.sync.dma_start(d_out[st], ot[:])
        es_d3.close()
        es_d.close()
        es_const.close()

    nc.compile()
    return nc


def _prep(hidden_states, cos, sin, attention_mask, ln1_w, ln2_w,
          Wq, Wk, Wv, Wo, Wg, Wu, Wd):
    """Host-side layout prep (slicing/transpose/pack/cast only)."""
    f32 = np.float32
    bf = ml_dtypes.bfloat16
    x = np.ascontiguousarray(np.asarray(hidden_states, dtype=f32))
    mask = np.broadcast_to(
        np.asarray(attention_mask, dtype=f32), (1, 1, S, S))[0, 0]

    wqg = np.ascontiguousarray(np.asarray(Wq, dtype=f32)[:, NH * HD:])
    wv_p = np.ascontiguousarray(np.asarray(Wv, dtype=f32).reshape(NT, 128, 512))
    wk_p = np.ascontiguousarray(np.asarray(Wk, dtype=f32).reshape(NT, 128, 512))
    wqg_p = np.ascontiguousarray(
        wqg.reshape(NT, 128, NT, 128).transpose(2, 1, 0, 3))
    wo_p = np.ascontiguousarray(np.asarray(Wo, dtype=f32).reshape(NT, 128, HID))
    wg_p = np.ascontiguousarray(
        np.asarray(Wg, dtype=f32).reshape(NT, 128, NFT, 128)
        .transpose(2, 1, 0, 3)).astype(bf)
    wu_p = np.ascontiguousarray(
        np.asarray(Wu, dtype=f32).reshape(NT, 128, NFT, 128)
        .transpose(2, 1, 0, 3)).astype(bf)
    wd_p = np.ascontiguousarray(
        np.asarray(Wd, dtype=f32).reshape(NFT, 128, HID)).astype(bf)

    ln1t = np.ascontiguousarray(np.asarray(ln1_w, dtype=f32).reshape(NT, 128).T)
    ln2t = np.ascontiguousarray(np.asarray(ln2_w, dtype=f32).reshape(NT, 128).T)
    onec = np.ones((128, 1), dtype=f32)
    oner = np.ones((1, 128), dtype=f32)
    ident = np.eye(128, dtype=f32)

    cosn = np.asarray(cos, dtype=f32)
    sinn = np.asarray(sin, dtype=f32)

    in_maps = []
    for core in range(8):
        b, q = core // 4, core % 4
        order = [(q + i) % 4 for i in range(4)]
        tperm = np.concatenate(
            [np.arange(o * SB, (o + 1) * SB) for o in order])
        xT = np.ascontiguousarray(x[b].T)            # [c, t]
        xT4 = xT.reshape(NT, 128, 4, SB)             # [ct, p, tb, t]
        xT_p = np.ascontiguousarray(xT4[:, :, order, :].transpose(2, 0, 1, 3))
        xq_p = np.ascontiguousarray(
            x[b, q * SB:(q + 1) * SB].reshape(4, 128, HID))
        mask_p = np.ascontiguousarray(
            (mask[q * SB:(q + 1) * SB][:, tperm] / CSC).reshape(4, 128, S))
        cA = np.tile(cosn[b, :, 0:32][:, None, :], (1, NKV, 1)).reshape(S, 128)
        cB = np.tile(cosn[b, :, 32:64][:, None, :], (1, NKV, 1)).reshape(S, 128)
        sA = np.tile(sinn[b, :, 0:32][:, None, :], (1, NKV, 1)).reshape(S, 128)
        sB = np.tile(sinn[b, :, 32:64][:, None, :], (1, NKV, 1)).reshape(S, 128)
        cA = np.ascontiguousarray(cA[tperm].reshape(16, 128, 128))
        cB = np.ascontiguousarray(cB[tperm].reshape(16, 128, 128))
        sA = np.ascontiguousarray(sA[tperm].reshape(16, 128, 128))
        sB = np.ascontiguousarray(sB[tperm].reshape(16, 128, 128))
        in_maps.append(dict(
            xT=xT_p, xq=xq_p, maskq=mask_p,
            cosA=cA, cosB=cB, sinA=sA, sinB=sB,
            ln1t=ln1t, ln2t=ln2t, onec=onec, oner=oner, ident=ident,
            wvp=wv_p, wkp=wk_p, wqgp=wqg_p, wop=wo_p,
            wgp=wg_p, wup=wu_p, wdp=wd_p,
        ))
    return in_maps


last_exec_time_ns = None


def kernel(hidden_states, cos, sin, attention_mask, ln1_w, ln2_w,
           Wq, Wk, Wv, Wo, Wg, Wu, Wd):
    global last_exec_time_ns
    if "nc" not in _cache:
        _cache["nc"] = _build()
    nc = _cache["nc"]
    in_maps = _prep(hidden_states, cos, sin, attention_mask, ln1_w, ln2_w,
                    Wq, Wk, Wv, Wo, Wg, Wu, Wd)
    trace = bool(os.environ.get("BASS_TRACE"))
    res = run_bass_kernel_spmd(nc, in_maps, list(range(8)), trace=trace)
    last_exec_time_ns = res.exec_time_ns
    out = np.empty((B, S, HID), dtype=np.float32)
    for core in range(8):
        b, q = core // 4, core % 4
        out[b, q * SB:(q + 1) * SB] = res.results[core]["out"].reshape(SB, HID)
    return out
